# revision 29
# baseline (speedup 1.0000x reference)
"""Trainium2 Bass kernel for nn_GaussianKernel (embedding_lookup / ridge).

Computation (per batch b of 16, N=256 tokens, K=128 RBF centers, H=16 out):
    gamma = gamma_table[tok_i, tok_j]; beta = beta_table[tok_i, tok_j]
    s     = gamma * d + beta                                  (B,N,N)
    psi_k = exp(-((s-mu_k)^2)/(2 sigma_k^2)) / (sqrt(2pi) sigma_k)
    h     = relu(psi @ W1 + b1); phi = h @ W2 + b2            (B,N,N,H)
    out   = transpose -> (B,H,N,N)

Key observation: phi is a fixed 16-vector-valued function of the single
scalar s.  On the host we least-squares fit phi(s) over the reachable
s-range in a 128-row piecewise-linear basis
    row0 = 1, row1 = u, row m = relu(u - k_m),   u = s - s_lo >= 0
(fit rel-RMS ~8e-4, far inside the 2e-2 gate; b2 and the -s_lo shift are
folded into the fitted coefficients).  The device then needs only, per
512-pair slab of flattened u:
    E = slopes^T @ u_flat      one f32r matmul (broadcast u to 128 rows)
    R = relu(E + bias_m)       ACT or DVE (alternating), per-partition bias
    P = coefC_t^T @ R          one f32r accumulating matmul; slab t of a
                               group of 8 lands on PSUM partitions 16t..16t+15
                               via zero-padded shifted stationaries, so one
                               [128,512] copy + one 256KB DMA drain 8 slabs.
float32r matmuls run 4x faster than float32 (1 cycle/row at >=256 cols);
f32r is tf32-like (~2e-4 rel rounding) which is far inside our budget.

When both tables are constant (the common case: gamma==g0, beta==b0), the
gather is folded into the host fit (slopes *= g0, bias += slope*(b0-s_lo))
and u IS d: pq tiles are staged straight from DRAM, no setup chain at all.
Otherwise a general gather path computes u = gamma*d + (beta - s_lo) via
one-hot matmuls, overlapped with the previous batch's slab loop.
"""

import numpy as np

import concourse.bass as bass
import concourse.mybir as mybir
import concourse.tile as tile
from concourse import bacc
from concourse.bass import ds
from concourse.bass_utils import run_bass_kernel_spmd

B, N, T, K, H = 16, 256, 128, 128, 16
NCORES = 8
BPC = B // NCORES          # batches per core
F32 = mybir.dt.float32
F32R = mybir.dt.float32r
AF = mybir.ActivationFunctionType
ALU = mybir.AluOpType

M = 128                    # basis rows (const + linear + 126 relu knots)
CW = 1540                  # packed const tile width
NSLAB = N * N // 512       # 128 slabs of 512 pairs per batch
NGRP = NSLAB // 8          # 16 groups of 8 slabs
ACT_SHARE = 67             # of every 92 E-path relus, this many go to ACT


def _build_nc(trivial):
    nc = bacc.Bacc("TRN2", target_bir_lowering=False)

    d_in = nc.dram_tensor("d", [BPC, N, N], F32R if trivial else F32,
                          kind="ExternalInput")
    if not trivial:
        tokf = nc.dram_tensor("tokf", [BPC, N], F32R, kind="ExternalInput")
    c_d = nc.dram_tensor("consts", [128, CW], F32R, kind="ExternalInput")
    out_d = nc.dram_tensor("out", [BPC, H, N, N], F32, kind="ExternalOutput")

    with tile.TileContext(nc) as tc:
        with (
            tc.tile_pool(name="consts", bufs=1) as cpool,
            tc.tile_pool(name="setup", bufs=2) as spool,
            tc.tile_pool(name="upool", bufs=2) as upool,
            tc.tile_pool(name="pairs", bufs=3) as ppool,
            tc.tile_pool(name="work", bufs=8) as wpool,
            tc.tile_pool(name="outp", bufs=3) as opool,
            tc.tile_pool(name="bcast", bufs=6) as bpool,
            tc.tile_pool(name="ps_e", bufs=5, space="PSUM") as ps_e,
            tc.tile_pool(name="ps_c", bufs=3, space="PSUM") as ps_c,
        ):
            # ---- constants: critical slice (slopes/bias) first ----
            C = cpool.tile([128, CW], F32R)
            nc.sync.dma_start(out=C[:, 1280:CW], in_=c_d[:, 1280:CW])
            if not trivial:
                nc.sync.dma_start(out=C[:, 0:256], in_=c_d[:, 0:256])
            nc.sync.dma_start(out=C[:, 256:1280], in_=c_d[:, 256:1280])
            gT_sb = C[:, 0:128]
            bT_sb = C[:, 128:256]
            cC8_sb = C[:, 256:1280]        # 8x[128,128] shifted coefC blocks
            slope_sb = C[0:1, 1280:1408]   # [1, 128] basis slopes
            ones_sb = C[0:1, 1408:1536]    # [1, 128] ones
            iota_sb = C[:, 1536:1537].bitcast(F32)
            bias_sb = C[:, 1537:1538].bitcast(F32)  # [128,1] relu bias

            # warm-up: each engine touches C once (absorbs the const DMA-lane
            # wait; Matmult instructions can hold only ONE sync wait)
            wus = cpool.tile([1, 16], F32)
            nc.vector.tensor_scalar(
                out=wus[:, 0:8], in0=C[0:1, 1280:1288], scalar1=0.0,
                scalar2=None, op0=ALU.add,
            )
            nc.scalar.copy(out=wus[:, 8:16], in_=C[0:1, 1280:1288])
            wu = ps_e.tile([1, 8], F32, tag="e")
            nc.tensor.matmul(wu, C[0:1, 1280:1281].bitcast(F32),
                             C[0:1, 1280:1288].bitcast(F32),
                             start=True, stop=True)
            nc.vector.tensor_scalar(
                out=wus[:, 0:8], in0=wu, scalar1=0.0, scalar2=None,
                op0=ALU.add,
            )

            # ---- per-batch setup (general path), split into parts so batch
            # bb+1's setup interleaves with batch bb's group loop ----
            def setup1(bb):
                ctx = {"bb": bb, "pq": {}, "u": []}
                if trivial:
                    return ctx
                tok_sb = spool.tile([1, N], F32R, tag="tok")
                nc.sync.dma_start(out=tok_sb, in_=tokf[bb : bb + 1, :])
                tb_ps = ps_e.tile([T, N], F32, tag="e")
                nc.tensor.matmul(tb_ps, ones_sb, tok_sb, start=True, stop=True)
                ot_sb = spool.tile([T, N], F32R, tag="ot")
                nc.vector.tensor_scalar(
                    out=ot_sb, in0=tb_ps, scalar1=iota_sb, scalar2=None,
                    op0=ALU.is_equal,
                )
                ctx["ot"] = ot_sb
                return ctx

            def setup2(ctx):
                if trivial:
                    return
                ot_sb = ctx["ot"]
                ag_ps = ps_e.tile([T, N], F32, tag="e")
                nc.tensor.matmul(ag_ps, gT_sb, ot_sb, start=True, stop=True)
                ag_sb = spool.tile([T, N], F32R, tag="ag")
                nc.scalar.copy(out=ag_sb, in_=ag_ps)
                ab_ps = ps_e.tile([T, N], F32, tag="e")
                nc.tensor.matmul(ab_ps, bT_sb, ot_sb, start=True, stop=True)
                ab_sb = spool.tile([T, N], F32R, tag="ab")
                nc.scalar.copy(out=ab_sb, in_=ab_ps)
                ctx["ag"] = ag_sb
                ctx["ab"] = ab_sb

            def setup3(ctx, hh):
                if trivial:
                    return
                bb, ot_sb = ctx["bb"], ctx["ot"]
                rows = ds(128 * hh, 128)
                dh_sb = spool.tile([128, N], F32, tag="d")
                nc.sync.dma_start(
                    out=dh_sb, in_=d_in[bb, 128 * hh : 128 * hh + 128, :]
                )
                g_ps = ps_e.tile([128, N], F32, tag="e")
                nc.tensor.matmul(g_ps, ot_sb[:, rows], ctx["ag"],
                                 start=True, stop=True)
                bt_ps = ps_e.tile([128, N], F32, tag="e")
                nc.tensor.matmul(bt_ps, ot_sb[:, rows], ctx["ab"],
                                 start=True, stop=True)
                u_sb = upool.tile([128, N], F32R)
                nc.vector.tensor_tensor(
                    out=u_sb, in0=dh_sb, in1=g_ps, op=ALU.mult
                )
                nc.vector.tensor_tensor(
                    out=u_sb, in0=u_sb, in1=bt_ps, op=ALU.add
                )
                ctx["u"].append(u_sb)

            def stage(ctx, e):
                pt = ppool.tile([1, 32 * N], F32R, name="pq")
                if trivial:
                    nc.sync.dma_start(
                        out=pt, in_=d_in[ctx["bb"], 32 * e : 32 * e + 32, :]
                    )
                else:
                    hh, qq = divmod(e, 4)
                    nc.sync.dma_start(
                        out=pt, in_=ctx["u"][hh][ds(32 * qq, 32), :]
                    )
                ctx["pq"][e] = pt

            relu_k_box = [0]               # global relu-pass counter
            pair_k = [0]                   # global A-pair slot counter
            ctx = setup1(0)
            setup2(ctx)
            setup3(ctx, 0)
            setup3(ctx, 1)
            stage(ctx, 0)
            stage(ctx, 1)

            # quad kinds: 'P' = Pool partition_broadcast, 'D' = DMA broadcast
            # straight from DRAM d (trivial path only), 'E' = E-matmul path.
            # Mix chosen so PE, Pool and the DMA engines all land ~74us.
            def make_qkind(counts):
                acc = {k: 0.0 for k in counts}
                total = sum(counts.values())
                out = []
                for q in range(total):
                    for k in counts:
                        acc[k] += counts[k] / total
                    pick = max(acc, key=lambda k: acc[k])
                    acc[pick] -= 1.0
                    out.append(pick)
                return out

            QKIND = make_qkind({"P": 25, "D": 16, "E": 23})
            d_flat = d_in.rearrange("b i j -> b (i j)") if trivial else None

            # front half of a group: broadcasts + E-matmuls + relus.
            # Emitted one group AHEAD of the C-matmul accumulation so a
            # slow producer chain never blocks PE's in-order queue.
            def emit_front(fctx, g):
                pq = fctx["pq"][g // 2]
                info = [None] * 8          # per slab: (r_tile, col_off)
                for half in range(2):
                    t0 = 4 * half
                    sl = (8 * g + t0) % 16  # slab within pq
                    kq = QKIND[pair_k[0] % 64] if trivial else "E"
                    pair_k[0] += 1
                    if kq in ("P", "D"):
                        ub = bpool.tile([128, 2048], F32R, name="ub")
                        if kq == "P":
                            nc.gpsimd.partition_broadcast(
                                ub, pq[:, ds(512 * sl, 2048)]
                            )
                        else:
                            src = d_flat[
                                fctx["bb"] : fctx["bb"] + 1,
                                ds(512 * (8 * g + t0), 2048),
                            ].broadcast_to([128, 2048])
                            nc.sync.dma_start(out=ub, in_=src)
                        r2 = wpool.tile([128, 2048], F32R, tag="r2",
                                        bufs=4)
                        nc.vector.tensor_scalar(
                            out=r2, in0=ub, scalar1=bias_sb,
                            scalar2=0.0, op0=ALU.add, op1=ALU.max,
                        )
                        for j in range(4):
                            info[t0 + j] = (r2, 512 * j)
                        continue
                    for t in range(t0, t0 + 4):
                        sl = (8 * g + t) % 16
                        e_ps = ps_e.tile([M, 512], F32, tag="e")
                        nc.tensor.matmul(
                            e_ps, slope_sb,
                            pq[:, ds(512 * sl, 512)],
                            start=True, stop=True,
                        )
                        r_sb = wpool.tile([M, 512], F32R, bufs=6)
                        if (relu_k_box[0] * ACT_SHARE) % 92 < ACT_SHARE:
                            nc.scalar.activation(
                                out=r_sb, in_=e_ps, func=AF.Relu,
                                bias=bias_sb,
                            )
                        else:
                            nc.vector.tensor_scalar(
                                out=r_sb, in0=e_ps, scalar1=bias_sb,
                                scalar2=0.0, op0=ALU.add, op1=ALU.max,
                            )
                        relu_k_box[0] += 1
                        info[t] = (r_sb, 0)
                return info

            front = emit_front(ctx, 0)

            for bb in range(BPC):
                out_flat = out_d[bb].rearrange("h i j -> h (i j)")
                nxt_ctx = None

                for g in range(NGRP):
                    nxt = g // 2 + 2
                    if g % 2 == 0 and nxt < 8:
                        stage(ctx, nxt)
                    if bb + 1 < BPC:
                        if g == 6:
                            nxt_ctx = setup1(bb + 1)
                        elif g == 8:
                            setup2(nxt_ctx)
                        elif g == 10:
                            setup3(nxt_ctx, 0)
                        elif g == 11:
                            setup3(nxt_ctx, 1)
                        elif g == 13:
                            stage(nxt_ctx, 0)
                        elif g == 14:
                            stage(nxt_ctx, 1)
                    # emit the NEXT group's producer chains first
                    if g + 1 < NGRP:
                        nfront = emit_front(ctx, g + 1)
                    elif bb + 1 < BPC:
                        nfront = emit_front(nxt_ctx, 0)
                    else:
                        nfront = None
                    # accumulate this group's 8 slabs into one PSUM bank:
                    # slab t lands on partitions 16t..16t+15 via a shifted
                    # zero-padded stationary
                    ogc = ps_c.tile([128, 512], F32, tag="c")
                    for t in range(8):
                        r_tile, coff = front[t]
                        nc.tensor.matmul(
                            ogc, cC8_sb[:, ds(128 * t, 128)],
                            r_tile[:, ds(coff, 512)],
                            start=(t == 0), stop=(t == 7),
                        )
                    front = nfront
                    og = opool.tile([128, 512], F32)
                    nc.scalar.activation(out=og, in_=ogc, func=AF.Copy)
                    dst = out_flat[:, ds(4096 * g, 4096)].rearrange(
                        "h (t c) -> t h c", t=8
                    )
                    nc.sync.dma_start(out=dst, in_=og[:, :])
                ctx = nxt_ctx
    nc.compile()
    return nc


_NC_CACHE = {}


def _get_nc(trivial=True):
    if trivial not in _NC_CACHE:
        _NC_CACHE[trivial] = _build_nc(trivial)
    return _NC_CACHE[trivial]


def _softplus(x):
    return np.logaddexp(0.0, x)


def kernel(d, tokens, mu, log_sigma, W1, b1, W2, b2, gamma_table, beta_table):
    d = np.ascontiguousarray(np.asarray(d), dtype=np.float32)
    d = np.nan_to_num(d, nan=0.0, posinf=0.0, neginf=0.0)
    tokens = np.asarray(tokens)
    mu = np.asarray(mu, dtype=np.float64)
    log_sigma = np.asarray(log_sigma, dtype=np.float64)
    W1 = np.asarray(W1, dtype=np.float64)
    b1 = np.asarray(b1, dtype=np.float64)
    W2 = np.asarray(W2, dtype=np.float64)
    b2 = np.asarray(b2, dtype=np.float64)
    gamma_table = np.asarray(gamma_table, dtype=np.float64)
    beta_table = np.asarray(beta_table, dtype=np.float64)

    g0 = float(gamma_table.flat[0])
    b0 = float(beta_table.flat[0])
    trivial = bool(np.all(gamma_table == g0) and np.all(beta_table == b0))

    # reachable s range: s = gamma*d + beta over table entries x [dmin,dmax]
    dmin = float(d.min())
    dmax = float(d.max())
    cand = np.stack([gamma_table * dmin + beta_table,
                     gamma_table * dmax + beta_table])
    s_lo = float(cand.min())
    s_hi = float(cand.max())
    R = max(s_hi - s_lo, 1e-6)

    sigma = _softplus(log_sigma) + 1e-6
    G = 8192

    def true_phi(sv):
        x = (sv[:, None] - mu) / sigma
        psi = np.exp(-0.5 * x * x) / (np.sqrt(2.0 * np.pi) * sigma)
        hmid = np.maximum(psi @ W1 + b1, 0.0)
        return hmid @ W2 + b2

    if trivial:
        # fit phi(g0*d + b0) directly in d-space: every basis row is
        # relu(d + q_m) (slopes all 1), so the broadcast fast path needs
        # no per-row slope at all
        Rd = max(dmax - dmin, 1e-6)
        dg = np.linspace(dmin, dmax, G)
        phig = true_phi(g0 * dg + b0)                  # [G, H]
        knots = dmin + Rd * np.arange(1, M - 1) / (M - 1)
        q = np.concatenate([[1.0 - dmin, -dmin], -knots])   # [M]
        A = np.maximum(dg[:, None] + q, 0.0)           # [G, M]
        coefC, *_ = np.linalg.lstsq(A, phig, rcond=None)
        slopes = np.ones(M)
        biases = q
    else:
        # general path: fit in u-space, u = gamma*d + (beta - s_lo) >= 0
        sg = np.linspace(s_lo, s_hi, G)
        phig = true_phi(sg)
        ug = sg - s_lo
        knots = R * np.arange(1, M - 1) / (M - 1)      # 126 interior knots
        A = np.concatenate(
            [np.ones((G, 1)), ug[:, None],
             np.maximum(ug[:, None] - knots, 0.0)], axis=1)  # [G, M]
        coefC, *_ = np.linalg.lstsq(A, phig, rcond=None)
        slopes = np.concatenate([[0.0, 1.0], np.ones(M - 2)])
        biases = np.concatenate([[1.0, 0.0], -knots])

    Cc = np.zeros((128, CW), dtype=np.float32)
    Cc[:, 0:128] = gamma_table.T.astype(np.float32)
    Cc[:, 128:256] = (beta_table - s_lo).T.astype(np.float32)
    cf = coefC.astype(np.float32)
    for t in range(8):                      # block t: coefC at cols 16t..+16
        base = 256 + 128 * t + 16 * t
        Cc[:, base : base + 16] = cf
    Cc[0, 1280:1408] = slopes.astype(np.float32)
    Cc[0, 1408:1536] = 1.0
    Cc[:, 1536] = np.arange(T, dtype=np.float32)
    Cc[:, 1537] = biases.astype(np.float32)

    common = {"consts": Cc}
    in_maps = []
    for c in range(NCORES):
        m = dict(common)
        m["d"] = np.ascontiguousarray(d[BPC * c : BPC * (c + 1)])
        if not trivial:
            m["tokf"] = np.ascontiguousarray(
                tokens.astype(np.float32)[BPC * c : BPC * (c + 1)])
        in_maps.append(m)

    nc = _get_nc(trivial)
    res = run_bass_kernel_spmd(nc, in_maps, list(range(NCORES))).results
    out = np.concatenate([res[c]["out"] for c in range(NCORES)], axis=0)
    return out.astype(np.float32)


# revision 32
# speedup vs baseline: 1.5210x; 1.5210x over previous
"""Trainium2 Bass kernel for nn_GaussianKernel (embedding_lookup / ridge).

Computation (per batch b of 16, N=256 tokens, K=128 RBF centers, H=16 out):
    gamma = gamma_table[tok_i, tok_j]; beta = beta_table[tok_i, tok_j]
    s     = gamma * d + beta                                  (B,N,N)
    psi_k = exp(-((s-mu_k)^2)/(2 sigma_k^2)) / (sqrt(2pi) sigma_k)
    h     = relu(psi @ W1 + b1); phi = h @ W2 + b2            (B,N,N,H)
    out   = transpose -> (B,H,N,N)

Key observation: phi is a fixed 16-vector-valued function of the single
scalar s.  On the host we least-squares fit phi(s) over the reachable
s-range in a 128-row piecewise-linear basis
    row0 = 1, row1 = u, row m = relu(u - k_m),   u = s - s_lo >= 0
(fit rel-RMS ~8e-4, far inside the 2e-2 gate; b2 and the -s_lo shift are
folded into the fitted coefficients).  The device then needs only, per
512-pair slab of flattened u:
    E = slopes^T @ u_flat      one f32r matmul (broadcast u to 128 rows)
    R = relu(E + bias_m)       ACT or DVE (alternating), per-partition bias
    P = coefC_t^T @ R          one f32r accumulating matmul; slab t of a
                               group of 8 lands on PSUM partitions 16t..16t+15
                               via zero-padded shifted stationaries, so one
                               [128,512] copy + one 256KB DMA drain 8 slabs.
float32r matmuls run 4x faster than float32 (1 cycle/row at >=256 cols);
f32r is tf32-like (~2e-4 rel rounding) which is far inside our budget.

When both tables are constant (the common case: gamma==g0, beta==b0), the
gather is folded into the host fit (slopes *= g0, bias += slope*(b0-s_lo))
and u IS d: pq tiles are staged straight from DRAM, no setup chain at all.
Otherwise a general gather path computes u = gamma*d + (beta - s_lo) via
one-hot matmuls, overlapped with the previous batch's slab loop.
"""

import numpy as np

import concourse.bass as bass
import concourse.mybir as mybir
import concourse.tile as tile
from concourse import bacc
from concourse.bass import ds
from concourse.bass_utils import run_bass_kernel_spmd

B, N, T, K, H = 16, 256, 128, 128, 16
NCORES = 8
BPC = B // NCORES          # batches per core
F32 = mybir.dt.float32
F32R = mybir.dt.float32r
AF = mybir.ActivationFunctionType
ALU = mybir.AluOpType

M = 128                    # basis rows (const + linear + 126 relu knots)
CW = 1540                  # packed const tile width
NSLAB = N * N // 512       # 128 slabs of 512 pairs per batch
NGRP = NSLAB // 8          # 16 groups of 8 slabs
ACT_SHARE = 67             # of every 92 E-path relus, this many go to ACT


def _build_nc(trivial):
    nc = bacc.Bacc("TRN2", target_bir_lowering=False)

    d_in = nc.dram_tensor("d", [BPC, N, N], F32R if trivial else F32,
                          kind="ExternalInput")
    if not trivial:
        tokf = nc.dram_tensor("tokf", [BPC, N], F32R, kind="ExternalInput")
    c_d = nc.dram_tensor("consts", [128, CW], F32R, kind="ExternalInput")
    out_d = nc.dram_tensor("out", [BPC, H, N, N], F32, kind="ExternalOutput")

    with tile.TileContext(nc) as tc:
        with (
            tc.tile_pool(name="consts", bufs=1) as cpool,
            tc.tile_pool(name="setup", bufs=2) as spool,
            tc.tile_pool(name="upool", bufs=2) as upool,
            tc.tile_pool(name="pairs", bufs=3) as ppool,
            tc.tile_pool(name="work", bufs=8) as wpool,
            tc.tile_pool(name="outp", bufs=3) as opool,
            tc.tile_pool(name="bcast", bufs=2) as bpool,
            tc.tile_pool(name="ps_e", bufs=5, space="PSUM") as ps_e,
            tc.tile_pool(name="ps_c", bufs=3, space="PSUM") as ps_c,
        ):
            # ---- constants: critical slice (slopes/bias) first ----
            C = cpool.tile([128, CW], F32R)
            nc.sync.dma_start(out=C[:, 1280:CW], in_=c_d[:, 1280:CW])
            if not trivial:
                nc.sync.dma_start(out=C[:, 0:256], in_=c_d[:, 0:256])
            nc.sync.dma_start(out=C[:, 256:1280], in_=c_d[:, 256:1280])
            gT_sb = C[:, 0:128]
            bT_sb = C[:, 128:256]
            cC8_sb = C[:, 256:1280]        # 8x[128,128] shifted coefC blocks
            slope_sb = C[0:1, 1280:1408]   # [1, 128] basis slopes
            ones_sb = C[0:1, 1408:1536]    # [1, 128] ones
            iota_sb = C[:, 1536:1537].bitcast(F32)
            bias_sb = C[:, 1537:1538].bitcast(F32)  # [128,1] relu bias

            # warm-up: each engine touches C once (absorbs the const DMA-lane
            # wait; Matmult instructions can hold only ONE sync wait)
            wus = cpool.tile([1, 16], F32)
            nc.vector.tensor_scalar(
                out=wus[:, 0:8], in0=C[0:1, 1280:1288], scalar1=0.0,
                scalar2=None, op0=ALU.add,
            )
            nc.scalar.copy(out=wus[:, 8:16], in_=C[0:1, 1280:1288])
            wu = ps_e.tile([1, 8], F32, tag="e")
            nc.tensor.matmul(wu, C[0:1, 1280:1281].bitcast(F32),
                             C[0:1, 1280:1288].bitcast(F32),
                             start=True, stop=True)
            nc.vector.tensor_scalar(
                out=wus[:, 0:8], in0=wu, scalar1=0.0, scalar2=None,
                op0=ALU.add,
            )

            # ---- per-batch setup (general path), split into parts so batch
            # bb+1's setup interleaves with batch bb's group loop ----
            def setup1(bb):
                ctx = {"bb": bb, "pq": {}, "u": []}
                if trivial:
                    return ctx
                tok_sb = spool.tile([1, N], F32R, tag="tok")
                nc.sync.dma_start(out=tok_sb, in_=tokf[bb : bb + 1, :])
                tb_ps = ps_e.tile([T, N], F32, tag="e")
                nc.tensor.matmul(tb_ps, ones_sb, tok_sb, start=True, stop=True)
                ot_sb = spool.tile([T, N], F32R, tag="ot")
                nc.vector.tensor_scalar(
                    out=ot_sb, in0=tb_ps, scalar1=iota_sb, scalar2=None,
                    op0=ALU.is_equal,
                )
                ctx["ot"] = ot_sb
                return ctx

            def setup2(ctx):
                if trivial:
                    return
                ot_sb = ctx["ot"]
                ag_ps = ps_e.tile([T, N], F32, tag="e")
                nc.tensor.matmul(ag_ps, gT_sb, ot_sb, start=True, stop=True)
                ag_sb = spool.tile([T, N], F32R, tag="ag")
                nc.scalar.copy(out=ag_sb, in_=ag_ps)
                ab_ps = ps_e.tile([T, N], F32, tag="e")
                nc.tensor.matmul(ab_ps, bT_sb, ot_sb, start=True, stop=True)
                ab_sb = spool.tile([T, N], F32R, tag="ab")
                nc.scalar.copy(out=ab_sb, in_=ab_ps)
                ctx["ag"] = ag_sb
                ctx["ab"] = ab_sb

            def setup3(ctx, hh):
                if trivial:
                    return
                bb, ot_sb = ctx["bb"], ctx["ot"]
                rows = ds(128 * hh, 128)
                dh_sb = spool.tile([128, N], F32, tag="d")
                nc.sync.dma_start(
                    out=dh_sb, in_=d_in[bb, 128 * hh : 128 * hh + 128, :]
                )
                g_ps = ps_e.tile([128, N], F32, tag="e")
                nc.tensor.matmul(g_ps, ot_sb[:, rows], ctx["ag"],
                                 start=True, stop=True)
                bt_ps = ps_e.tile([128, N], F32, tag="e")
                nc.tensor.matmul(bt_ps, ot_sb[:, rows], ctx["ab"],
                                 start=True, stop=True)
                u_sb = upool.tile([128, N], F32R)
                nc.vector.tensor_tensor(
                    out=u_sb, in0=dh_sb, in1=g_ps, op=ALU.mult
                )
                nc.vector.tensor_tensor(
                    out=u_sb, in0=u_sb, in1=bt_ps, op=ALU.add
                )
                ctx["u"].append(u_sb)

            def stage(ctx, e):
                pt = ppool.tile([1, 32 * N], F32R, name="pq")
                if trivial:
                    nc.sync.dma_start(
                        out=pt, in_=d_in[ctx["bb"], 32 * e : 32 * e + 32, :]
                    )
                else:
                    hh, qq = divmod(e, 4)
                    nc.sync.dma_start(
                        out=pt, in_=ctx["u"][hh][ds(32 * qq, 32), :]
                    )
                ctx["pq"][e] = pt

            relu_k_box = [0]               # global relu-pass counter
            pair_k = [0]                   # global A-pair slot counter
            ctx = setup1(0)
            setup2(ctx)
            setup3(ctx, 0)
            setup3(ctx, 1)
            stage(ctx, 0)
            stage(ctx, 1)

            # quad kinds: 'P' = Pool partition_broadcast, 'D' = DMA broadcast
            # straight from DRAM d (trivial path only), 'E' = E-matmul path.
            # Mix chosen so PE, Pool and the DMA engines all land ~74us.
            def make_qkind(counts):
                acc = {k: 0.0 for k in counts}
                total = sum(counts.values())
                out = []
                for q in range(total):
                    for k in counts:
                        acc[k] += counts[k] / total
                    pick = max(acc, key=lambda k: acc[k])
                    acc[pick] -= 1.0
                    out.append(pick)
                return out

            QKIND = make_qkind({"P": 25, "D": 16, "E": 23})
            d_flat = d_in.rearrange("b i j -> b (i j)") if trivial else None

            def quad_kind(bbi, g, half):
                if not trivial:
                    return "E"
                return QKIND[(bbi * 2 * NGRP + g * 2 + half) % 64]

            # D-quad DMA broadcasts depend only on DRAM d: issue them
            # DBC_AHEAD groups early so the ~3us transfer latency is hidden
            # and PE never stalls (a PE stall also drops its p-state).
            dq_ub = {}

            def emit_dbcast(bbi, g):
                for half in range(2):
                    if quad_kind(bbi, g, half) != "D":
                        continue
                    t0 = 4 * half
                    ub = bpool.tile([128, 2048], F32R, name="ubd",
                                    tag="ubd", bufs=4)
                    src = d_flat[
                        bbi : bbi + 1, ds(512 * (8 * g + t0), 2048)
                    ].broadcast_to([128, 2048])
                    nc.sync.dma_start(out=ub, in_=src)
                    dq_ub[(bbi, g, half)] = ub

            # front half of a group: broadcasts + E-matmuls + relus.
            # Emitted one group AHEAD of the C-matmul accumulation so a
            # slow producer chain never blocks PE's in-order queue.
            def emit_front(fctx, g):
                pq = fctx["pq"][g // 2]
                bbi = fctx["bb"]
                info = [None] * 8          # per slab: (r_tile, col_off)
                for half in range(2):
                    t0 = 4 * half
                    sl = (8 * g + t0) % 16  # slab within pq
                    kq = quad_kind(bbi, g, half)
                    if kq in ("P", "D"):
                        if kq == "P":
                            ub = bpool.tile([128, 2048], F32R, name="ub")
                            nc.gpsimd.partition_broadcast(
                                ub, pq[:, ds(512 * sl, 2048)]
                            )
                        else:
                            ub = dq_ub.pop((bbi, g, half))
                        r2 = wpool.tile([128, 2048], F32R, tag="r2",
                                        bufs=4)
                        nc.vector.tensor_scalar(
                            out=r2, in0=ub, scalar1=bias_sb,
                            scalar2=0.0, op0=ALU.add, op1=ALU.max,
                        )
                        for j in range(4):
                            info[t0 + j] = (r2, 512 * j)
                        continue
                    for t in range(t0, t0 + 4):
                        sl = (8 * g + t) % 16
                        e_ps = ps_e.tile([M, 512], F32, tag="e")
                        nc.tensor.matmul(
                            e_ps, slope_sb,
                            pq[:, ds(512 * sl, 512)],
                            start=True, stop=True,
                        )
                        r_sb = wpool.tile([M, 512], F32R, bufs=6)
                        if (relu_k_box[0] * ACT_SHARE) % 92 < ACT_SHARE:
                            nc.scalar.activation(
                                out=r_sb, in_=e_ps, func=AF.Relu,
                                bias=bias_sb,
                            )
                        else:
                            nc.vector.tensor_scalar(
                                out=r_sb, in0=e_ps, scalar1=bias_sb,
                                scalar2=0.0, op0=ALU.add, op1=ALU.max,
                            )
                        relu_k_box[0] += 1
                        info[t] = (r_sb, 0)
                return info

            DBC_AHEAD = 4
            if trivial:
                for gg in range(DBC_AHEAD):
                    emit_dbcast(0, gg)
            front = emit_front(ctx, 0)

            for bb in range(BPC):
                out_flat = out_d[bb].rearrange("h i j -> h (i j)")
                nxt_ctx = None

                for g in range(NGRP):
                    nxt = g // 2 + 2
                    if g % 2 == 0 and nxt < 8:
                        stage(ctx, nxt)
                    if bb + 1 < BPC:
                        if g == 6:
                            nxt_ctx = setup1(bb + 1)
                        elif g == 8:
                            setup2(nxt_ctx)
                        elif g == 10:
                            setup3(nxt_ctx, 0)
                        elif g == 11:
                            setup3(nxt_ctx, 1)
                        elif g == 13:
                            stage(nxt_ctx, 0)
                        elif g == 14:
                            stage(nxt_ctx, 1)
                    # D-quad DMA broadcasts several groups ahead
                    if trivial:
                        ga = g + DBC_AHEAD
                        if ga < NGRP:
                            emit_dbcast(bb, ga)
                        elif bb + 1 < BPC:
                            emit_dbcast(bb + 1, ga - NGRP)
                    # emit the NEXT group's producer chains first
                    if g + 1 < NGRP:
                        nfront = emit_front(ctx, g + 1)
                    elif bb + 1 < BPC:
                        nfront = emit_front(nxt_ctx, 0)
                    else:
                        nfront = None
                    # accumulate this group's 8 slabs into one PSUM bank:
                    # slab t lands on partitions 16t..16t+15 via a shifted
                    # zero-padded stationary
                    ogc = ps_c.tile([128, 512], F32, tag="c")
                    for t in range(8):
                        r_tile, coff = front[t]
                        nc.tensor.matmul(
                            ogc, cC8_sb[:, ds(128 * t, 128)],
                            r_tile[:, ds(coff, 512)],
                            start=(t == 0), stop=(t == 7),
                        )
                    front = nfront
                    og = opool.tile([128, 512], F32)
                    nc.scalar.activation(out=og, in_=ogc, func=AF.Copy)
                    dst = out_flat[:, ds(4096 * g, 4096)].rearrange(
                        "h (t c) -> t h c", t=8
                    )
                    nc.sync.dma_start(out=dst, in_=og[:, :])
                ctx = nxt_ctx
    nc.compile()
    return nc


_NC_CACHE = {}


def _get_nc(trivial=True):
    if trivial not in _NC_CACHE:
        _NC_CACHE[trivial] = _build_nc(trivial)
    return _NC_CACHE[trivial]


def _softplus(x):
    return np.logaddexp(0.0, x)


def kernel(d, tokens, mu, log_sigma, W1, b1, W2, b2, gamma_table, beta_table):
    d = np.ascontiguousarray(np.asarray(d), dtype=np.float32)
    d = np.nan_to_num(d, nan=0.0, posinf=0.0, neginf=0.0)
    tokens = np.asarray(tokens)
    mu = np.asarray(mu, dtype=np.float64)
    log_sigma = np.asarray(log_sigma, dtype=np.float64)
    W1 = np.asarray(W1, dtype=np.float64)
    b1 = np.asarray(b1, dtype=np.float64)
    W2 = np.asarray(W2, dtype=np.float64)
    b2 = np.asarray(b2, dtype=np.float64)
    gamma_table = np.asarray(gamma_table, dtype=np.float64)
    beta_table = np.asarray(beta_table, dtype=np.float64)

    g0 = float(gamma_table.flat[0])
    b0 = float(beta_table.flat[0])
    trivial = bool(np.all(gamma_table == g0) and np.all(beta_table == b0))

    # reachable s range: s = gamma*d + beta over table entries x [dmin,dmax]
    dmin = float(d.min())
    dmax = float(d.max())
    cand = np.stack([gamma_table * dmin + beta_table,
                     gamma_table * dmax + beta_table])
    s_lo = float(cand.min())
    s_hi = float(cand.max())
    R = max(s_hi - s_lo, 1e-6)

    sigma = _softplus(log_sigma) + 1e-6
    G = 8192

    def true_phi(sv):
        x = (sv[:, None] - mu) / sigma
        psi = np.exp(-0.5 * x * x) / (np.sqrt(2.0 * np.pi) * sigma)
        hmid = np.maximum(psi @ W1 + b1, 0.0)
        return hmid @ W2 + b2

    if trivial:
        # fit phi(g0*d + b0) directly in d-space: every basis row is
        # relu(d + q_m) (slopes all 1), so the broadcast fast path needs
        # no per-row slope at all
        Rd = max(dmax - dmin, 1e-6)
        dg = np.linspace(dmin, dmax, G)
        phig = true_phi(g0 * dg + b0)                  # [G, H]
        knots = dmin + Rd * np.arange(1, M - 1) / (M - 1)
        q = np.concatenate([[1.0 - dmin, -dmin], -knots])   # [M]
        A = np.maximum(dg[:, None] + q, 0.0)           # [G, M]
        coefC, *_ = np.linalg.lstsq(A, phig, rcond=None)
        slopes = np.ones(M)
        biases = q
    else:
        # general path: fit in u-space, u = gamma*d + (beta - s_lo) >= 0
        sg = np.linspace(s_lo, s_hi, G)
        phig = true_phi(sg)
        ug = sg - s_lo
        knots = R * np.arange(1, M - 1) / (M - 1)      # 126 interior knots
        A = np.concatenate(
            [np.ones((G, 1)), ug[:, None],
             np.maximum(ug[:, None] - knots, 0.0)], axis=1)  # [G, M]
        coefC, *_ = np.linalg.lstsq(A, phig, rcond=None)
        slopes = np.concatenate([[0.0, 1.0], np.ones(M - 2)])
        biases = np.concatenate([[1.0, 0.0], -knots])

    Cc = np.zeros((128, CW), dtype=np.float32)
    Cc[:, 0:128] = gamma_table.T.astype(np.float32)
    Cc[:, 128:256] = (beta_table - s_lo).T.astype(np.float32)
    cf = coefC.astype(np.float32)
    for t in range(8):                      # block t: coefC at cols 16t..+16
        base = 256 + 128 * t + 16 * t
        Cc[:, base : base + 16] = cf
    Cc[0, 1280:1408] = slopes.astype(np.float32)
    Cc[0, 1408:1536] = 1.0
    Cc[:, 1536] = np.arange(T, dtype=np.float32)
    Cc[:, 1537] = biases.astype(np.float32)

    common = {"consts": Cc}
    in_maps = []
    for c in range(NCORES):
        m = dict(common)
        m["d"] = np.ascontiguousarray(d[BPC * c : BPC * (c + 1)])
        if not trivial:
            m["tokf"] = np.ascontiguousarray(
                tokens.astype(np.float32)[BPC * c : BPC * (c + 1)])
        in_maps.append(m)

    nc = _get_nc(trivial)
    res = run_bass_kernel_spmd(nc, in_maps, list(range(NCORES))).results
    out = np.concatenate([res[c]["out"] for c in range(NCORES)], axis=0)
    return out.astype(np.float32)
